# revision 33
# baseline (speedup 1.0000x reference)
"""Trainium2 Bass kernel for EquivariantPPFAttention (gnn_message_passing).

Contract: kernel(**inputs) takes FULL unsharded inputs (as produced by
reference.setup_inputs()) and returns the FULL [N, OUT, 3] float32 output.

Strategy (data-parallel over query points N across 8 NeuronCores):
  - shard q_pts / neighbor_indices across cores; replicate everything else.
  - one combined gather table comb[M, 512B]: s_feats row in bf16 (384B) +
    s_pts/normals in f32 (24B) + pad. Per query-tile of 128, dma_gather
    pulls all 128*32 neighbor rows (4 gathers of 1024 idxs - HW limit).
    The 4 gathers of a tile ride SWDGE queues 0-3 (num_swdge_queues=4):
    queue q's descriptor gen runs on Q7 core pair q, so the four gathers'
    descriptor generation runs CONCURRENTLY (~4x SWDGE throughput; this
    was the dominant baseline cost).
  - K-sum of the bf16 s_feats part on DVE (tree adds, f32 final);
    pts/normals extracted to a packed buffer for the PPF phase.
  - PPF angles without cross products: |a x b|^2 = |a|^2|b|^2 - (a.b)^2,
    then atan2(r,y) = atan(r*recip(y)) + pi*[y<0]. All elementwise work
    fused over the 3-component axis and the three angles share one wide
    DVE tail (per-op overhead dominated at small tiles otherwise).
  - tiny MLP on TensorE with rows on the free dim; two query-tiles packed
    per matmul via block-diagonal weights; mean-over-K folded into W3; the
    1/pi PPF normalization folded into W1; 1/K of the value path folded
    into Wv. MLP K-reduce via f16 tree adds (2x DVE mode).
  - emission is chunked (CT tiles) with gather+K-sum one chunk AHEAD of
    PPF/MLP: engine queues execute in order, so the K-sum of chunk c+1
    must sit before chunk c's MLP in the DVE queue or gathers stall.
  - output written f16 (host casts to f32; halves output DMA bytes), via
    the two HWDGE rings (SP/ACT); Pool queues are reserved for gathers.
"""

import math
import os
import numpy as np

N = 20000
M = 20000
K = 32
D = 64
HID = 64
OUT = 192
PPF_OUT = 64
N_CORES = 8
PI = math.pi

ES = 128          # f32 elems per comb row (512 B)
SFW = 96          # f32 slots holding the 192 bf16 s_feats values
PNO = 96          # f32 slot offset of pts/normals (6 floats)
NI = 1024         # idxs per dma_gather (HW-stable limit)
GPT = (128 * K) // NI   # gathers per query tile (4)
KPG = K // GPT    # k-blocks per gather (8)

_NC_CACHE = {}


def _build_nc(T, stage="full", loop=None):
    """Per-core Bass program for T query-tiles of 128.

    stage: debug bisection point - gather | ppf | mlp | full.
    loop: if set, repeat the whole body N times (for timing).
    """
    from contextlib import ExitStack, nullcontext
    from concourse import bacc, bass, mybir, tile

    assert T % 2 == 0
    NPAIR = T // 2
    NQ = 128 * T
    f32 = mybir.dt.float32
    bf16 = mybir.dt.bfloat16
    f16 = mybir.dt.float16
    i16 = mybir.dt.int16
    AF = mybir.ActivationFunctionType
    ALU = mybir.AluOpType

    nc = bacc.Bacc(
        "TRN2", target_bir_lowering=False, debug=False, num_swdge_queues=4
    )

    comb_in = nc.dram_tensor("comb", [M, ES], f32, kind="ExternalInput")
    qp_in = nc.dram_tensor("qp", [128, T, 3], f16, kind="ExternalInput")
    idx_in = nc.dram_tensor("idx16", [128, T, GPT, NI // 16], i16,
                            kind="ExternalInput")
    w1b_in = nc.dram_tensor("w1b", [8, 128], f16, kind="ExternalInput")
    b1b_in = nc.dram_tensor("b1b", [128, 1], f32, kind="ExternalInput")
    w2b_in = nc.dram_tensor("w2b", [128, 128], f16, kind="ExternalInput")
    b2b_in = nc.dram_tensor("b2b", [128, 1], f32, kind="ExternalInput")
    w3b_in = nc.dram_tensor("w3b", [128, 128], f32, kind="ExternalInput")
    b3b_in = nc.dram_tensor("b3b", [128, 1], f32, kind="ExternalInput")
    wgb_in = nc.dram_tensor("wgb", [128, 3, 128], f32, kind="ExternalInput")
    bgb_in = nc.dram_tensor("bgb", [128, 3], f32, kind="ExternalInput")
    wvb_in = nc.dram_tensor("wvb", [128, 3, 128], f32, kind="ExternalInput")
    ident_in = nc.dram_tensor("ident", [128, 128], f32, kind="ExternalInput")

    SIMPLEOUT = bool(int(os.environ.get("BENCH_SIMPLEOUT", "0")))
    if stage in ("full", "mlponly"):
        if SIMPLEOUT:
            out_dev = nc.dram_tensor(
                "out", [128, 9 * NQ], f32, kind="ExternalOutput"
            )
        else:
            out_dev = nc.dram_tensor(
                "out", [3, OUT, NQ], f16, kind="ExternalOutput"
            )
        dbg = None
    else:
        DBGW = {
            "gather": T * K * 8 + T * 192,
            "ppf": 4 * T * K,
            "mlp": 128 * K + 128 + 128 + 3 * 128,
            "gonly": T * 8,
            "gred": T * 8,
            "dbgdma": T * K * 8 + T * 192,
        }[stage]
        out_dev = None
        dbg = nc.dram_tensor("dbg", [128, DBGW], f32, kind="ExternalOutput")

    with tile.TileContext(nc) as tc, ExitStack() as ctx:
        const = ctx.enter_context(tc.tile_pool(name="const", bufs=1))
        gpool = ctx.enter_context(tc.tile_pool(name="gpool", bufs=2))
        gath = ctx.enter_context(tc.tile_pool(name="gath", bufs=1))
        planes = ctx.enter_context(tc.tile_pool(name="planes", bufs=1))
        temps = ctx.enter_context(tc.tile_pool(name="temps", bufs=2))
        mlpp = ctx.enter_context(tc.tile_pool(name="mlpp", bufs=1))
        small = ctx.enter_context(tc.tile_pool(name="small", bufs=2))
        psmlp = ctx.enter_context(tc.tile_pool(name="psmlp", bufs=2, space="PSUM"))
        pssm = ctx.enter_context(tc.tile_pool(name="pssm", bufs=2, space="PSUM"))
        pstp = ctx.enter_context(tc.tile_pool(name="pstp", bufs=2, space="PSUM"))

        def cload(name, dram, shape, dt=f32):
            t = const.tile(shape, dt, tag=name, name=name)
            if len(shape) > 3:
                dims = " ".join(f"d{i}" for i in range(len(shape) - 1))
                pat = f"p {dims} -> p ({dims})"
                nc.sync.dma_start(t[:].rearrange(pat), dram.ap().rearrange(pat))
            else:
                nc.sync.dma_start(t[:], dram.ap())
            return t

        qp_t = cload("qp", qp_in, [128, T, 3], f16)
        idx_t = cload("idx16", idx_in, [128, T, GPT, NI // 16], i16)
        w1b_t = cload("w1b", w1b_in, [8, 128], f16)
        b1b_t = cload("b1b", b1b_in, [128, 1])
        w2b_t = cload("w2b", w2b_in, [128, 128], f16)
        b2b_t = cload("b2b", b2b_in, [128, 1])
        w3b_t = cload("w3b", w3b_in, [128, 128])
        b3b_t = cload("b3b", b3b_in, [128, 1])
        wgb_t = cload("wgb", wgb_in, [128, 3, 128])
        bgb_t = cload("bgb", bgb_in, [128, 3])
        wvb_t = cload("wvb", wvb_in, [128, 3, 128])
        ident_t = cload("ident", ident_in, [128, 128])

        _loop_ctx = tc.For_i(0, loop, 1) if loop else nullcontext()
        with _loop_ctx:
            # ---- gather + per-tile K-reduce + pn extraction ----
            nbbuf = gath.tile([128, T, K, 8], f16, tag="nbbuf")
            sfsum = gath.tile([128, T, 192], f32, tag="sfsum")

            do_gather = stage not in ("dbgdma", "mlponly")
            do_reduce = stage not in ("gonly", "dbgdma", "mlponly")

            if stage in ("dbgdma", "mlponly"):
                nc.vector.memset(nbbuf[:].rearrange("p t k c -> p (t k c)"), 0.25)
                nc.vector.memset(sfsum[:].rearrange("p t c -> p (t c)"), 0.25)

            NORED = bool(int(os.environ.get("BENCH_NORED", "0")))
            if NORED and stage == "full":
                nc.vector.memset(nbbuf[:].rearrange("p t k c -> p (t k c)"), 0.25)
                nc.vector.memset(sfsum[:].rearrange("p t c -> p (t c)"), 0.25)

            gt_last = None
            gts = {}

            def emit_gather(t):
                nonlocal gt_last
                gt = gpool.tile([128, K, ES], f32, tag="gt", name="gt", bufs=4)
                gt_last = gt
                gts[t] = gt
                for g in range(GPT):
                    # queue g -> Q7 core pair g: the 4 gathers of one
                    # tile run concurrently on the 4 SWDGE core pairs.
                    nc.gpsimd.dma_gather(
                        out_ap=gt[:, g * KPG : (g + 1) * KPG, :],
                        in_ap=comb_in.ap(),
                        idxs_ap=idx_t[:, t, g, :],
                        num_idxs=NI,
                        num_idxs_reg=NI,
                        elem_size=ES,
                        queue_num=g,
                    )

            def emit_reduce(t):
                gt = gts.pop(t)
                if do_reduce and not NORED:
                    # K-sum of the bf16 s_feats block via a contiguous
                    # tree-add (f16 intermediates, f32 final level)
                    gtb = gt[:].bitcast(bf16)          # [128, K, 256]
                    ga = gpool.tile([128, 16, 256], f16, tag="ga", bufs=2)
                    nc.vector.tensor_tensor(
                        ga[:, :, 0:192], gtb[:, 0:16, 0:192],
                        gtb[:, 16:32, 0:192], ALU.add,
                    )
                    for lv in (8, 4, 2):
                        nc.vector.tensor_tensor(
                            ga[:, 0:lv, 0:192], ga[:, 0:lv, 0:192],
                            ga[:, lv : 2 * lv, 0:192], ALU.add,
                        )
                    nc.vector.tensor_tensor(
                        sfsum[:, t, :], ga[:, 0, 0:192], ga[:, 1, 0:192],
                        ALU.add,
                    )
                    # pts/normals (f32) -> nbbuf[:, t, :, 0:6]; on DVE so
                    # the ACT queue carries no gather-dependent ops.
                    nc.vector.tensor_copy(
                        nbbuf[:, t, :, 0:6], gt[:, :, PNO : PNO + 6]
                    )

            def emit_gather_tile(t):
                emit_gather(t)
                emit_reduce(t)

            if do_gather and stage != "full":
                for t in range(T):
                    emit_gather_tile(t)
            if stage in ("gonly", "gred"):
                sb = gath.tile([128, T * 8], f32, tag="sdbg")
                nc.vector.tensor_copy(
                    sb[:],
                    nbbuf[:, 0, :, :].rearrange("p k c -> p (k c)")[:, : T * 8],
                )
                nc.sync.dma_start(dbg.ap()[:, : T * 8], sb[:])
            elif stage in ("gather", "dbgdma"):
                nc.sync.dma_start(
                    dbg.ap()[:, : T * K * 8],
                    nbbuf[:].rearrange("p t k c -> p (t k c)"),
                )
                nc.sync.dma_start(
                    dbg.ap()[:, T * K * 8 :],
                    sfsum[:].rearrange("p t c -> p (t c)"),
                )
            else:
                # ---- PPF + MLP, emitted in tile chunks so gather/PPF of
                # chunk c+1 overlaps the MLP pipeline of chunk c ----
                FW = T * K
                TT = nc.vector.tensor_tensor
                STT = nc.vector.scalar_tensor_tensor

                # interleaved PPF planes: p4[:, t, ci, k] (ci-major per tile)
                p4 = planes.tile([128, T, 4, K], f32, tag="p4", name="p4")

                def plane_view(ci):
                    # [128, FW] view of feature ci (strided; debug stages only)
                    return p4[:, :, ci, :].rearrange("p t k -> p (t k)")

                RW = 128 * K  # rows per query-tile (4096)
                HC = RW // 2
                NOPACK = bool(int(os.environ.get("BENCH_NOPACK", "0")))
                NOVAL = bool(int(os.environ.get("BENCH_NOVAL", "0")))
                NOGATE = bool(int(os.environ.get("BENCH_NOGATE", "0")))
                NOMLP12 = bool(int(os.environ.get("BENCH_NOMLP12", "0")))
                V1PACK = bool(int(os.environ.get("BENCH_V1PACK", "0")))
                PACKNODMA = bool(int(os.environ.get("BENCH_PACKNODMA", "0")))
                PACKNOTP = bool(int(os.environ.get("BENCH_PACKNOTP", "0")))

                if stage in ("full", "mlponly") and not SIMPLEOUT:
                    out_re = out_dev.ap().rearrange(
                        "c (jj p) q -> p c jj q", jj=3
                    )

                def emit_ppf(t0, tn):
                    with nc.allow_low_precision(reason="f16 PPF chain"):
                        _emit_ppf(t0, tn)

                def _emit_ppf(t0, tn):
                    # |a x b|^2 = |a|^2 |b|^2 - (a.b)^2 : no cross products.
                    # All elementwise work is fused over the 3-component axis
                    # ([128, 3, tn, K] tiles) and the three angles share one
                    # [128, 3*tn*K] tail, minimizing DVE per-op overhead.
                    nb = nbbuf[:, t0 : t0 + tn]
                    TS = nc.vector.tensor_scalar
                    E = tn * K

                    def ctile(tag):
                        return temps.tile([128, 3, tn, K], f16, tag=tag,
                                          name=tag)

                    def etile(tag):
                        return temps.tile([128, tn * K], f16, tag=tag,
                                          name=tag)

                    # component-major strided views of the gathered pn data
                    np3 = nb[:, :, :, 0:3].rearrange("p t k c -> p c t k")
                    nn3 = nb[:, :, :, 3:6].rearrange("p t k c -> p c t k")
                    qn3 = nb[:, :, 0, 3:6].rearrange(
                        "p t c -> p c t"
                    ).to_broadcast([128, 3, tn, K])
                    qp3 = qp_t[:, t0 : t0 + tn, :].rearrange(
                        "p t c -> p c t"
                    ).to_broadcast([128, 3, tn, K])

                    vd3 = ctile("vd3")
                    TT(vd3[:], np3, qp3, ALU.subtract)

                    # products, then reduce over the component axis
                    y3x = temps.tile([128, 3, E], f16, tag="y3x", name="y3x")
                    msq3x = temps.tile([128, 3, E], f16, tag="msq3x",
                                       name="msq3x")
                    dd = etile("dd")
                    nnsq = etile("nnsq")

                    def dotred(prod_tag, a, b, out):
                        pr = ctile(prod_tag)
                        TT(pr[:], a, b, ALU.mult)
                        nc.vector.reduce_sum(
                            out,
                            pr[:].rearrange("p c t k -> p (t k) c"),
                            mybir.AxisListType.X,
                        )

                    dotred("pr0", vd3[:], vd3[:], dd[:])
                    dotred("pr1", qn3, vd3[:], y3x[:, 0, :])
                    dotred("pr0", nn3, vd3[:], y3x[:, 1, :])
                    dotred("pr1", qn3, nn3, y3x[:, 2, :])
                    dotred("pr0", nn3, nn3, nnsq[:])

                    # |qn|^2 per query [128, tn]
                    qn_c = nb[:, :, 0, 3:6]
                    pq = temps.tile([128, tn, 3], f16, tag="pq")
                    TT(pq[:], qn_c, qn_c, ALU.mult)
                    qn_sq = temps.tile([128, tn], f16, tag="qnsq")
                    nc.vector.reduce_sum(qn_sq[:], pq[:], mybir.AxisListType.X)
                    qnsq_b = qn_sq[:].to_broadcast([128, tn, K])

                    ddv = dd[:].rearrange("p (t k) -> p t k", k=K)
                    nnsqv = nnsq[:].rearrange("p (t k) -> p t k", k=K)
                    m0 = msq3x[:, 0, :].rearrange("p (t k) -> p t k", k=K)
                    m1 = msq3x[:, 1, :].rearrange("p (t k) -> p t k", k=K)
                    m2 = msq3x[:, 2, :].rearrange("p (t k) -> p t k", k=K)
                    TT(m0, qnsq_b, ddv, ALU.mult)
                    TT(m1, nnsqv, ddv, ALU.mult)
                    TT(m2, qnsq_b, nnsqv, ALU.mult)

                    # fused tail over all three angles at once
                    y3f = y3x[:].rearrange("p c e -> p (c e)")
                    m3f = msq3x[:].rearrange("p c e -> p (c e)")
                    ysq3 = temps.tile([128, 3 * E], f16, tag="ysq3")
                    TT(ysq3[:], y3f, y3f, ALU.mult)
                    TT(m3f, m3f, ysq3[:], ALU.subtract)
                    TS(m3f, m3f, 0.0, None, ALU.max)

                    # atan2(r, y) = pi/2 - atan(y/r) for r >= 0 -- branch-free
                    # (also right at r=0: recip(+0)=+inf -> atan = +-pi/2).
                    nc.scalar.activation(
                        p4[:, t0 : t0 + tn, 0, :], ddv, AF.Sqrt
                    )
                    r3 = temps.tile([128, 3 * E], f16, tag="r3")
                    nc.scalar.activation(r3[:], m3f, AF.Sqrt)
                    ir3 = temps.tile([128, 3 * E], f16, tag="ir3")
                    nc.vector.reciprocal(ir3[:], r3[:])
                    tq3 = temps.tile([128, 3 * E], f16, tag="tq3")
                    TT(tq3[:], y3f, ir3[:], ALU.mult)
                    at3 = temps.tile([128, 3 * E], f16, tag="at3")
                    nc.scalar.activation(at3[:], tq3[:], AF.Arctan)
                    at3v = at3[:].rearrange("p (c t k) -> p c t k", c=3, k=K)
                    for ci in range(3):
                        TS(
                            p4[:, t0 : t0 + tn, ci + 1, :],
                            at3v[:, ci], -1.0, PI / 2.0,
                            ALU.mult, ALU.add,
                        )

                def emit_pair(j):
                    pf = mlpp.tile([8, RW], f16, tag="pf", bufs=2)
                    if NOPACK:
                        nc.vector.memset(pf[:], 0.25)
                    elif V1PACK:
                        for t2 in range(2):
                            t_abs = 2 * j + t2
                            for ci in range(4):
                                nc.sync.dma_start(
                                    pf[t2 * 4 + ci : t2 * 4 + ci + 1, :],
                                    p4[:, t_abs, ci, :],
                                )
                    else:
                        # pack via PE transpose: p4 tile slab [128 q, (ci k)]
                        # -> [(ci k), 128 q]; then 4 row-DMAs per tile with
                        # 512B descriptors, spread over 3 DMA-issue engines.
                        pts_s = mlpp.tile([128, 2, 128], f16, tag="pts", bufs=3)
                        if PACKNOTP:
                            nc.vector.memset(
                                pts_s[:].rearrange("p t q -> p (t q)"), 0.25
                            )
                        else:
                            for t2 in range(2):
                                t_abs = 2 * j + t2
                                tp = pstp.tile([128, 128], f32, tag="pstp")
                                nc.tensor.transpose(
                                    tp[:],
                                    p4[:, t_abs, :, :].rearrange(
                                        "p c k -> p (c k)"
                                    ),
                                    ident_t[:],
                                )
                                nc.scalar.activation(
                                    pts_s[:, t2, :], tp[:], AF.Copy
                                )
                        if PACKNODMA:
                            nc.vector.memset(pf[:], 0.25)
                        else:
                            # one DMA per tile: dst [4, 4096] row-major ==
                            # src [128, 128] partition-major traversal
                            # (dst col k*128+q <- src partition ci*32+k col q).
                            # Pool queues are reserved for the gathers now, so
                            # packs ride the two HWDGE rings (SP/ACT).
                            engs = [nc.sync, nc.scalar]
                            for t2 in range(2):
                                eng = engs[(j * 2 + t2) % 2]
                                eng.dma_start(
                                    pf[t2 * 4 : (t2 + 1) * 4, :],
                                    pts_s[:, t2, :],
                                )

                    # value-path transposes depend only on sfsum: run them
                    # on PE/ACT while the h1/h2 stream occupies the pipeline
                    if stage != "mlp" and not NOVAL:
                        av_e = sfsum[:, 2 * j : 2 * j + 2, :].rearrange(
                            "p t (d c) -> p c (t d)", c=3
                        )
                        aggs3 = small.tile([128, 3, 128], f32, tag="aggs3")
                        for c in range(3):
                            tpv = pssm.tile([128, 128], f32, tag="pssm")
                            nc.tensor.transpose(tpv[:], av_e[:, c, :], ident_t[:])
                            nc.scalar.activation(aggs3[:, c, :], tpv[:], AF.Copy)

                    ksum = small.tile([128, 128], f32, tag="ksum")
                    if NOMLP12:
                        nc.vector.memset(ksum[:], 0.25)
                    # h1 for both halves first (8 consecutive w1b matmuls ->
                    # one LDWEIGHTS), then h2 (8 consecutive w2b matmuls);
                    # ReLUs batched 1024-wide to amortize ACT fixed cost.
                    h1list = []
                    for hh in range(0 if NOMLP12 else 2):
                        h1s = mlpp.tile([128, HC], f16, tag="h1s", bufs=2)
                        for half in range(2):
                            h1p = psmlp.tile([128, 1024], f32, tag="psmlp")
                            for sub in range(2):
                                co = half * 1024 + sub * 512
                                nc.tensor.matmul(
                                    h1p[:, sub * 512 : (sub + 1) * 512],
                                    w1b_t[:],
                                    pf[:, hh * HC + co : hh * HC + co + 512],
                                    start=True, stop=True,
                                )
                            nc.scalar.activation(
                                h1s[:, half * 1024 : (half + 1) * 1024],
                                h1p[:], AF.Relu, bias=b1b_t[:],
                            )
                        h1list.append(h1s)
                    kparts = []
                    for hh in range(0 if NOMLP12 else 2):
                        h1s = h1list[hh]
                        h2s = mlpp.tile([128, HC], f16, tag="h2s", bufs=2)
                        for half in range(2):
                            h2p = psmlp.tile([128, 1024], f32, tag="psmlp")
                            for sub in range(2):
                                co = half * 1024 + sub * 512
                                nc.tensor.matmul(
                                    h2p[:, sub * 512 : (sub + 1) * 512],
                                    w2b_t[:],
                                    h1s[:, co : co + 512],
                                    start=True, stop=True,
                                )
                            nc.scalar.activation(
                                h2s[:, half * 1024 : (half + 1) * 1024],
                                h2p[:], AF.Relu, bias=b2b_t[:],
                            )
                        # cols are k-major (col = k*128 + q): each hh holds
                        # k-block hh*16..hh*16+15 for all 128 queries.
                        # K-reduce via f16 tree adds (2x DVE mode) instead of
                        # tensor_reduce (1x mode).
                        h2v = h2s[:].rearrange("p (k q) -> p k q", q=128)
                        for lv in (8, 4, 2):
                            TT(
                                h2v[:, 0:lv, :], h2v[:, 0:lv, :],
                                h2v[:, lv : 2 * lv, :], ALU.add,
                            )
                        kp = small.tile([128, 128], f32, tag=f"kp{hh}")
                        TT(kp[:], h2v[:, 0, :], h2v[:, 1, :], ALU.add)
                        kparts.append(kp)
                    if not NOMLP12:
                        TT(ksum[:], kparts[0][:], kparts[1][:], ALU.add)

                    pmp = pssm.tile([128, 128], f32, tag="pssm")
                    nc.tensor.matmul(pmp[:], w3b_t[:], ksum[:], start=True, stop=True)
                    pms = small.tile([128, 128], f32, tag="pms")
                    nc.vector.tensor_scalar_add(pms[:], pmp[:], b3b_t[:])

                    if stage == "mlp" and j == 0:
                        nc.sync.dma_start(dbg.ap()[:, : RW // 2], h2s[:])
                        nc.sync.dma_start(dbg.ap()[:, RW : RW + 128], ksum[:])
                        nc.sync.dma_start(dbg.ap()[:, RW + 128 : RW + 256], pms[:])

                    gates = []
                    for jj in range(3):
                        gs = small.tile(
                            [128, 128], f16, tag=f"gate{jj}", name=f"gate{jj}"
                        )
                        if NOGATE:
                            nc.vector.memset(gs[:], 0.5)
                            gates.append(gs)
                            continue
                        gp = pssm.tile([128, 128], f32, tag="pssm")
                        nc.tensor.matmul(
                            gp[:], wgb_t[:, jj, :], pms[:], start=True, stop=True
                        )
                        nc.scalar.activation(
                            gs[:], gp[:], AF.Sigmoid, bias=bgb_t[:, jj : jj + 1]
                        )
                        gates.append(gs)
                        if stage == "mlp" and j == 0:
                            nc.sync.dma_start(
                                dbg.ap()[
                                    :,
                                    RW + 256 + jj * 128 : RW + 256 + (jj + 1) * 128,
                                ],
                                gs[:],
                            )
                    if stage == "mlp":
                        return

                    # value path: one transpose per component covers both
                    # tiles of the pair:
                    # in [128 q, (2 t x 64 d)] -> out [(2 t x 64 d), 128 q]
                    vstage = small.tile([128, 3, 3, 128], f16, tag="vstage")
                    if NOVAL:
                        nc.vector.memset(
                            vstage[:].rearrange("p a b c -> p (a b c)"), 0.25
                        )
                    for jj in range(0 if NOVAL else 3):
                        # one matmul covers all 3 spatial components (free
                        # dim 384) instead of 3x 128-wide; PSUM staged to
                        # f16 SBUF via ACT so the gate multiplies run in the
                        # DVE 2x mode instead of 1x PSUM-source mode.
                        vp3 = psmlp.tile([128, 1024], f32, tag="psmlp")
                        nc.tensor.matmul(
                            vp3[:, 0:384],
                            wvb_t[:, jj, :],
                            aggs3[:].rearrange("p a b -> p (a b)"),
                            start=True, stop=True,
                        )
                        vs3 = small.tile([128, 384], f16, tag="vs3")
                        nc.scalar.activation(vs3[:], vp3[:, 0:384], AF.Copy)
                        for c in range(3):
                            TT(
                                vstage[:, c, jj, :],
                                vs3[:, c * 128 : (c + 1) * 128],
                                gates[jj][:], ALU.mult,
                            )

                    for h in range(2):
                        q0 = (2 * j + h) * 128
                        if SIMPLEOUT:
                            nc.sync.dma_start(
                                out_dev.ap()[0:64, q0 * 9 : q0 * 9 + 9 * 128],
                                vstage[h * 64 : (h + 1) * 64, :, :, :].rearrange(
                                    "p c jj q -> p (c jj q)"
                                ),
                            )
                        else:
                            eng = nc.scalar if h == 0 else nc.sync
                            eng.dma_start(
                                out_re[:, :, :, q0 : q0 + 128].rearrange(
                                    "p c jj q -> p (c jj) q"
                                ),
                                vstage[h * 64 : (h + 1) * 64, :, :, :].rearrange(
                                    "p c jj q -> p (c jj) q"
                                ),
                            )

                if stage == "mlponly":
                    nc.vector.memset(
                        p4[:].rearrange("p t c k -> p (t c k)"), 0.25
                    )
                    for j in range(NPAIR):
                        emit_pair(j)
                elif stage == "ppf":
                    emit_ppf(0, T)
                    for ci in range(4):
                        nc.sync.dma_start(
                            dbg.ap()[:, ci * FW : (ci + 1) * FW], plane_view(ci)
                        )
                elif stage == "mlp":
                    emit_ppf(0, T)
                    emit_pair(0)
                else:
                    # per-chunk pipeline, ordered to keep every in-order
                    # engine queue stall-free: iteration c emits
                    #   [gathers c+1] [ppf c] [trees+pn c+1] [pairs c]
                    # so the DVE queue is ppf(c) -> trees(c+1) -> pairs(c):
                    # while the pair chain's PE/ACT/DMA legs run, the DVE
                    # chews the next chunk's K-sums instead of stalling at
                    # the pair ops' head, and pair DVE ops are ready by the
                    # time the queue reaches them.
                    CT = int(os.environ.get("BENCH_CT", "4"))
                    assert T % CT == 0 and CT % 2 == 0
                    chunks = list(range(0, T, CT))
                    if do_gather:
                        for t in range(chunks[0], chunks[0] + CT):
                            emit_gather(t)
                        for t in range(chunks[0], chunks[0] + CT):
                            emit_reduce(t)
                    for ci, t0 in enumerate(chunks):
                        nxt = (
                            range(chunks[ci + 1], chunks[ci + 1] + CT)
                            if ci + 1 < len(chunks) else []
                        )
                        if do_gather:
                            for t in nxt:
                                emit_gather(t)
                        emit_ppf(t0, CT)
                        if do_gather:
                            for t in nxt:
                                emit_reduce(t)
                        for j in range(t0 // 2, (t0 + CT) // 2):
                            emit_pair(j)

    nc.compile()
    return nc


def _f32_to_bf16_bits(x):
    """Round-to-nearest-even f32 -> bf16, returned as uint16 bits."""
    u = np.ascontiguousarray(x, dtype=np.float32).view(np.uint32)
    rounded = (u + 0x7FFF + ((u >> 16) & 1)) >> 16
    return rounded.astype(np.uint16)


def _host_prep(q_pts, s_pts, s_feats, neighbor_indices, normals,
               W1, b1, W2, b2, W3, b3, Wg, bg, Wv, T, n_total=N):
    NQ = 128 * T
    n_per_core = n_total // N_CORES
    f = np.float32

    comb = np.zeros((M, ES), dtype=f)
    cb = comb.view(np.uint16).reshape(M, ES * 2)
    cb[:, : 2 * SFW] = _f32_to_bf16_bits(s_feats.reshape(M, 192))
    comb[:, PNO : PNO + 3] = s_pts
    comb[:, PNO + 3 : PNO + 6] = normals

    W1T = W1.T.astype(f).copy()
    W1T[1:4] *= f(1.0 / PI)
    w1b = np.zeros((8, 128), dtype=f)
    w1b[0:4, 0:64] = W1T
    w1b[4:8, 64:128] = W1T
    b1b = np.concatenate([b1, b1]).astype(f)[:, None]

    def blockdiag2(A):
        n_, m_ = A.shape
        o = np.zeros((2 * n_, 2 * m_), dtype=f)
        o[:n_, :m_] = A
        o[n_:, m_:] = A
        return o

    w2b = blockdiag2(W2.T.astype(f))
    b2b = np.concatenate([b2, b2]).astype(f)[:, None]
    w3b = blockdiag2((W3.T / K).astype(f))
    b3b = np.concatenate([b3, b3]).astype(f)[:, None]

    WgT = Wg.T.astype(f)
    WvT = (Wv.T / K).astype(f)
    wgb = np.zeros((3, 128, 128), dtype=f)
    wvb = np.zeros((3, 128, 128), dtype=f)
    bgb = np.zeros((128, 3), dtype=f)
    for jj in range(3):
        wgb[jj] = blockdiag2(WgT[:, jj * 64 : (jj + 1) * 64])
        wvb[jj] = blockdiag2(WvT[:, jj * 64 : (jj + 1) * 64])
        bgb[:, jj] = np.concatenate([bg[jj * 64 : (jj + 1) * 64]] * 2)
    wgb_host = np.ascontiguousarray(wgb.transpose(1, 0, 2))
    wvb_host = np.ascontiguousarray(wvb.transpose(1, 0, 2))
    ident = np.eye(128, dtype=f)

    shared = dict(
        comb=comb, w1b=w1b.astype(np.float16), b1b=b1b,
        w2b=w2b.astype(np.float16), b2b=b2b, w3b=w3b, b3b=b3b,
        wgb=wgb_host, bgb=bgb, wvb=wvb_host, ident=ident,
    )

    in_maps = []
    for i in range(N_CORES):
        lo = i * n_per_core
        hi = lo + n_per_core
        qp_pad = np.zeros((NQ, 3), dtype=f)
        qp_pad[: hi - lo] = q_pts[lo:hi]
        idx_pad = np.zeros((NQ, K), dtype=np.int64)
        idx_pad[: hi - lo] = neighbor_indices[lo:hi]

        qp_host = np.ascontiguousarray(
            qp_pad.reshape(T, 128, 3).transpose(1, 0, 2)
        ).astype(np.float16)

        # idx16[p, t, g, s]: gather g of tile t covers logical rows
        # i' = (k - g*KPG)*128 + q, wrapped: w[l, s] = list[s*16 + l]
        idx16 = np.zeros((128, T, GPT, NI // 16), np.int16)
        for t in range(T):
            arr = idx_pad[t * 128 : (t + 1) * 128, :]      # [128 q, K]
            for g in range(GPT):
                lst = arr[:, g * KPG : (g + 1) * KPG].T.reshape(NI)
                idx16[:, t, g, :] = np.tile(
                    lst.reshape(NI // 16, 16).T.astype(np.int16), (8, 1)
                )

        m = dict(shared)
        m.update(qp=qp_host, idx16=idx16)
        in_maps.append(m)
    return in_maps


def kernel(**inputs):
    from concourse.bass_utils import run_bass_kernel_spmd

    T = 20
    inputs = {k: np.asarray(v) for k, v in inputs.items()}
    idx = inputs["neighbor_indices"].astype(np.int64)

    if T not in _NC_CACHE:
        _NC_CACHE[T] = _build_nc(T)
    nc = _NC_CACHE[T]

    in_maps = _host_prep(
        inputs["q_pts"], inputs["s_pts"], inputs["s_feats"], idx,
        inputs["normals"], inputs["W1"], inputs["b1"], inputs["W2"],
        inputs["b2"], inputs["W3"], inputs["b3"], inputs["Wg"],
        inputs["bg"], inputs["Wv"], T,
    )
    res = run_bass_kernel_spmd(nc, in_maps, core_ids=list(range(N_CORES)))

    n_per_core = N // N_CORES
    out = np.empty((N, OUT, 3), dtype=np.float32)
    for i in range(N_CORES):
        o = res.results[i]["out"]
        out[i * n_per_core : (i + 1) * n_per_core] = o.transpose(2, 1, 0)[:n_per_core]
    return out

